# revision 1
# baseline (speedup 1.0000x reference)
# Involution2d (K=7) Trainium2 kernel — 8-core SPMD, batch+spatial sharding.
#
# Sharding: 8 cores = (batch b in 0..3) x (H-half in 0..1). Each core computes
# a [128, 32, 64] output block. Per core, on device:
#   1. kernel generation: 1x1 conv (BN folded) -> ReLU -> 1x1 conv -> [49, 2048]
#      per-pixel involution kernels (+ bias, x boundary mask folded in).
#   2. involution: acc[c, p] = sum_o kerm[o, p] * xw[c, p + shift_o]
#      - ker rows broadcast across 128 partitions via K=1 matmuls on TensorE
#      - multiply/accumulate on VectorE; row shifts are free-dim AP offsets
#        into a halo-padded x layout; W-edge wraps are killed by the mask.
import numpy as np

EPS = 1e-5
KK = 7
C = 128
H = 64
W = 64
B = 4
HH = 32            # rows per core
P = HH * W         # 2048 output pixels per core
NIN = 4 + 38 * W + 4   # 2440: 3-row halos + 4-elem guard pads each side
GEN_CHUNK = 512
BC_CHUNK = 1024    # broadcast/psum chunk (2 PSUM banks)

_STATE = {}


def _build():
    import concourse.tile as tile
    from concourse import bacc, mybir

    f32 = mybir.dt.float32
    nc = bacc.Bacc("TRN2", target_bir_lowering=False, debug=False)

    xw_d = nc.dram_tensor("xw", [C, NIN], f32, kind="ExternalInput").ap()
    w1sT_d = nc.dram_tensor("w1sT", [C, 32], f32, kind="ExternalInput").ap()
    b1f_d = nc.dram_tensor("b1f", [32, 1], f32, kind="ExternalInput").ap()
    w2T_d = nc.dram_tensor("w2T", [32, 49], f32, kind="ExternalInput").ap()
    b2f_d = nc.dram_tensor("b2f", [49, 1], f32, kind="ExternalInput").ap()
    mask_d = nc.dram_tensor("maskt", [49, P], f32, kind="ExternalInput").ap()
    out_d = nc.dram_tensor("out", [C, P], f32, kind="ExternalOutput").ap()

    with tile.TileContext(nc) as tc:
        with (
            tc.tile_pool(name="consts", bufs=1) as cpool,
            tc.tile_pool(name="work", bufs=2) as wpool,
            tc.tile_pool(name="pgen", bufs=2, space="PSUM") as pgen,
            tc.tile_pool(name="pbc", bufs=2, space="PSUM") as pbc,
        ):
            x_sb = cpool.tile([C, NIN], f32, tag="x")
            nc.sync.dma_start(x_sb[:], xw_d)
            w1sT = cpool.tile([C, 32], f32, tag="w1")
            nc.sync.dma_start(w1sT[:], w1sT_d)
            b1f = cpool.tile([32, 1], f32, tag="b1")
            nc.sync.dma_start(b1f[:], b1f_d)
            w2T = cpool.tile([32, 49], f32, tag="w2")
            nc.sync.dma_start(w2T[:], w2T_d)
            b2f = cpool.tile([49, 1], f32, tag="b2")
            nc.sync.dma_start(b2f[:], b2f_d)
            mask_sb = cpool.tile([49, P], f32, tag="mask")
            nc.sync.dma_start(mask_sb[:], mask_d)
            ones_sb = cpool.tile([1, C], f32, tag="ones")
            nc.vector.memset(ones_sb[:], 1.0)

            f_sb = cpool.tile([32, P], f32, tag="f")
            kerm_sb = cpool.tile([49, P], f32, tag="kerm")
            acc_sb = cpool.tile([C, P], f32, tag="acc")

            # ---- kernel generation ----
            # x view for the core's own rows: starts 3 halo rows in (+4 guard)
            XOFF = 4 + 3 * W
            for ci in range(P // GEN_CHUNK):
                sl = slice(ci * GEN_CHUNK, (ci + 1) * GEN_CHUNK)
                xsl = slice(XOFF + ci * GEN_CHUNK, XOFF + (ci + 1) * GEN_CHUNK)
                f1 = pgen.tile([32, GEN_CHUNK], f32, tag="f1")
                nc.tensor.matmul(f1[:], w1sT[:], x_sb[:, xsl], start=True, stop=True)
                # f = relu(f1 + b1f)  (ScalarE, per-partition bias)
                nc.scalar.activation(
                    f_sb[:, sl], f1[:], mybir.ActivationFunctionType.Relu,
                    bias=b1f[:],
                )
                k2 = pgen.tile([49, GEN_CHUNK], f32, tag="k2")
                nc.tensor.matmul(k2[:], w2T[:], f_sb[:, sl], start=True, stop=True)
                # kerm = (k2 + b2) * mask  (VectorE fused)
                nc.vector.scalar_tensor_tensor(
                    out=kerm_sb[:, sl], in0=k2[:], scalar=b2f[:],
                    in1=mask_sb[:, sl],
                    op0=mybir.AluOpType.add, op1=mybir.AluOpType.mult,
                )

            # ---- involution accumulate ----
            NB = BC_CHUNK // 512
            for o in range(49):
                ip, jp = divmod(o, 7)
                A = W * ip + jp + 1
                # matmul rhs must start at partition 0 -> DMA ker row o there
                krow = wpool.tile([1, P], f32, tag="krow")
                nc.sync.dma_start(krow[:], kerm_sb[o:o + 1, :])
                for h2 in range(P // BC_CHUNK):
                    bc = pbc.tile([C, BC_CHUNK], f32, tag="bc")
                    base = h2 * BC_CHUNK
                    for nb in range(NB):
                        nc.tensor.matmul(
                            bc[:, nb * 512:(nb + 1) * 512],
                            ones_sb[:],
                            krow[0:1, base + nb * 512: base + (nb + 1) * 512],
                            start=True, stop=True,
                        )
                    xs = x_sb[:, A + base: A + base + BC_CHUNK]
                    osl = slice(base, base + BC_CHUNK)
                    if o == 0:
                        nc.vector.tensor_mul(acc_sb[:, osl], xs, bc[:])
                    else:
                        prod = wpool.tile([C, BC_CHUNK], f32, tag="prod")
                        nc.vector.tensor_mul(prod[:], xs, bc[:])
                        nc.vector.tensor_add(acc_sb[:, osl], acc_sb[:, osl], prod[:])

            nc.sync.dma_start(out_d, acc_sb[:])

    nc.compile()
    return nc


def _get_nc():
    if "nc" not in _STATE:
        _STATE["nc"] = _build()
    return _STATE["nc"]


def _host_prep(x, w1, b1, bn_gamma, bn_beta, bn_mean, bn_var, w2, b2):
    x = np.asarray(x, dtype=np.float32)
    scale = np.asarray(bn_gamma) / np.sqrt(np.asarray(bn_var) + EPS)
    w1s = (np.asarray(w1) * scale[:, None]).astype(np.float32)
    b1f = (np.asarray(b1) * scale + np.asarray(bn_beta)
           - np.asarray(bn_mean) * scale).astype(np.float32)
    w1sT = np.ascontiguousarray(w1s.T)                      # [128, 32]
    w2T = np.ascontiguousarray(np.asarray(w2, np.float32).T)  # [32, 49]
    b1fc = np.ascontiguousarray(b1f[:, None])               # [32, 1]
    b2fc = np.ascontiguousarray(np.asarray(b2, np.float32)[:, None])  # [49, 1]

    # W-edge mask: kerm[o, p] = 0 where w + dj leaves the row
    wcol = np.arange(P, dtype=np.int64) % W
    maskt = np.zeros((49, P), dtype=np.float32)
    for ipp in range(KK):
        for jpp in range(KK):
            dj = jpp - 3
            maskt[ipp * KK + jpp] = ((wcol + dj >= 0) & (wcol + dj < W))
    maskt = np.ascontiguousarray(maskt)

    in_maps = []
    for core in range(8):
        b, half = divmod(core, 2)
        h0 = HH * half
        xw = np.zeros((C, NIN), dtype=np.float32)
        lo = max(0, h0 - 3)
        hi = min(H, h0 + HH + 3)
        # rows [lo, hi) -> xw positions 4 + 64*(row - h0 + 3)
        src = x[b, :, lo:hi, :].reshape(C, -1)
        start = 4 + W * (lo - h0 + 3)
        xw[:, start:start + src.shape[1]] = src
        in_maps.append({
            "xw": xw, "w1sT": w1sT, "b1f": b1fc, "w2T": w2T,
            "b2f": b2fc, "maskt": maskt,
        })
    return in_maps


def run(inputs: dict, trace: bool = False):
    from concourse.bass_utils import run_bass_kernel_spmd

    nc = _get_nc()
    in_maps = _host_prep(**inputs)
    res = run_bass_kernel_spmd(
        nc, in_maps, core_ids=list(range(8)), trace=trace,
    )
    out = np.zeros((B, C, H, W), dtype=np.float32)
    for core in range(8):
        b, half = divmod(core, 2)
        h0 = HH * half
        out[b, :, h0:h0 + HH, :] = res.results[core]["out"].reshape(C, HH, W)
    return out, res


def kernel(**inputs) -> np.ndarray:
    out, _ = run(inputs, trace=False)
    return out



# revision 2
# speedup vs baseline: 2.5647x; 2.5647x over previous
# Involution2d (K=7) Trainium2 kernel — 8-core SPMD, batch+spatial sharding.
#
# V2: fp16 data path. The V1 bottleneck was fp32 broadcast matmuls (854 ns
# each, 335/375 us total). Changes:
#   - whole pipeline in fp16: moving-operand matmuls run 4x faster (1 cyc/col
#     vs 4), DVE tensor_tensor ops run in 2x_1P mode (16-bit packed).
#   - per-offset kerm-row DMAs replaced by one-hot stationary matmuls
#     (E_o^T @ kerm broadcasts row o to 128 partitions straight from kerm).
#   - ScalarE copies each PSUM broadcast to SBUF fp16 so both DVE ops keep
#     2x mode (any PSUM operand would cap tensor_tensor at 1x); this also
#     moves ~100us of work onto the otherwise-idle Scalar engine.
#   - dual (even/odd) halo'd x copies keep every DVE slice 4B-aligned, which
#     the 2x_1P packed mode requires.
# Sharding: 8 cores = (batch b in 0..3) x (H-half in 0..1), each core owns a
# [128, 32, 64] output block ([C, P=2048] on device).
import numpy as np

EPS = 1e-5
KK = 7
C = 128
H = 64
W = 64
B = 4
HH = 32            # rows per core
P = HH * W         # 2048 output pixels per core
NIN = 4 + 38 * W + 4   # 2440: 3-row halos + 4-elem guard pads each side
XOFF = 4 + 3 * W
GEN_CHUNK = 512
BC_CHUNK = 1024    # broadcast/psum chunk (2 PSUM banks)

_STATE = {}


def _build():
    import concourse.tile as tile
    from concourse import bacc, mybir

    f32 = mybir.dt.float32
    f16 = mybir.dt.float16
    nc = bacc.Bacc("TRN2", target_bir_lowering=False, debug=False)

    xwe_d = nc.dram_tensor("xwe", [C, NIN], f16, kind="ExternalInput").ap()
    xwo_d = nc.dram_tensor("xwo", [C, NIN], f16, kind="ExternalInput").ap()
    w1sT_d = nc.dram_tensor("w1sT", [C, 32], f16, kind="ExternalInput").ap()
    b1f_d = nc.dram_tensor("b1f", [32, 1], f32, kind="ExternalInput").ap()
    w2T_d = nc.dram_tensor("w2T", [32, 49], f16, kind="ExternalInput").ap()
    b2f_d = nc.dram_tensor("b2f", [49, 1], f32, kind="ExternalInput").ap()
    mask_d = nc.dram_tensor("maskt", [49, P], f16, kind="ExternalInput").ap()
    eye_d = nc.dram_tensor("eye", [49, 49 * 128], f16, kind="ExternalInput").ap()
    out_d = nc.dram_tensor("out", [C, P], f16, kind="ExternalOutput").ap()

    with tile.TileContext(nc) as tc:
        with (
            tc.tile_pool(name="consts", bufs=1) as cpool,
            tc.tile_pool(name="work", bufs=3) as wpool,
            tc.tile_pool(name="pgen", bufs=2, space="PSUM") as pgen,
            tc.tile_pool(name="pbc", bufs=2, space="PSUM") as pbc,
        ):
            xwe = cpool.tile([C, NIN], f16, tag="xe")
            nc.sync.dma_start(xwe[:], xwe_d)
            xwo = cpool.tile([C, NIN], f16, tag="xo")
            nc.sync.dma_start(xwo[:], xwo_d)
            w1sT = cpool.tile([C, 32], f16, tag="w1")
            nc.sync.dma_start(w1sT[:], w1sT_d)
            b1f = cpool.tile([32, 1], f32, tag="b1")
            nc.sync.dma_start(b1f[:], b1f_d)
            w2T = cpool.tile([32, 49], f16, tag="w2")
            nc.sync.dma_start(w2T[:], w2T_d)
            b2f = cpool.tile([49, 1], f32, tag="b2")
            nc.sync.dma_start(b2f[:], b2f_d)
            mask_sb = cpool.tile([49, P], f16, tag="mask")
            nc.sync.dma_start(mask_sb[:], mask_d)
            eye_sb = cpool.tile([49, 49 * 128], f16, tag="eye")
            nc.sync.dma_start(eye_sb[:], eye_d)

            f_sb = cpool.tile([32, P], f16, tag="f")
            kerm_sb = cpool.tile([49, P], f16, tag="kerm")
            acc_sb = cpool.tile([C, P], f16, tag="acc")

            # ---- kernel generation (fp16 in, fp32 psum) ----
            for ci in range(P // GEN_CHUNK):
                sl = slice(ci * GEN_CHUNK, (ci + 1) * GEN_CHUNK)
                xsl = slice(XOFF + ci * GEN_CHUNK, XOFF + (ci + 1) * GEN_CHUNK)
                f1 = pgen.tile([32, GEN_CHUNK], f32, tag="f1")
                nc.tensor.matmul(f1[:], w1sT[:], xwe[:, xsl], start=True, stop=True)
                nc.scalar.activation(
                    f_sb[:, sl], f1[:], mybir.ActivationFunctionType.Relu,
                    bias=b1f[:],
                )
                k2 = pgen.tile([49, GEN_CHUNK], f32, tag="k2")
                nc.tensor.matmul(k2[:], w2T[:], f_sb[:, sl], start=True, stop=True)
                # kerm = (k2 + b2) * mask  -> fp16
                nc.vector.scalar_tensor_tensor(
                    out=kerm_sb[:, sl], in0=k2[:], scalar=b2f[:],
                    in1=mask_sb[:, sl],
                    op0=mybir.AluOpType.add, op1=mybir.AluOpType.mult,
                )

            # ---- involution accumulate ----
            for o in range(49):
                ip, jp = divmod(o, 7)
                A = W * ip + jp + 1
                if A % 2 == 0:
                    xsrc, Ax = xwe, A
                else:
                    xsrc, Ax = xwo, A - 1
                esl = eye_sb[:, o * 128:(o + 1) * 128]
                for h2 in range(P // BC_CHUNK):
                    base = h2 * BC_CHUNK
                    bc = pbc.tile([C, BC_CHUNK], f32, tag="bc")
                    for nb in range(BC_CHUNK // 512):
                        nc.tensor.matmul(
                            bc[:, nb * 512:(nb + 1) * 512],
                            esl,
                            kerm_sb[:, base + nb * 512: base + (nb + 1) * 512],
                            start=True, stop=True,
                        )
                    bch = wpool.tile([C, BC_CHUNK], f16, tag="bch")
                    nc.scalar.activation(
                        bch[:], bc[:], mybir.ActivationFunctionType.Copy,
                    )
                    xs = xsrc[:, Ax + base: Ax + base + BC_CHUNK]
                    osl = slice(base, base + BC_CHUNK)
                    if o == 0:
                        nc.vector.tensor_mul(acc_sb[:, osl], xs, bch[:])
                    else:
                        prod = wpool.tile([C, BC_CHUNK], f16, tag="prod")
                        nc.vector.tensor_mul(prod[:], xs, bch[:])
                        nc.vector.tensor_add(acc_sb[:, osl], acc_sb[:, osl], prod[:])

            nc.sync.dma_start(out_d, acc_sb[:])

    nc.compile()
    return nc


def _get_nc():
    if "nc" not in _STATE:
        _STATE["nc"] = _build()
    return _STATE["nc"]


def _host_prep(x, w1, b1, bn_gamma, bn_beta, bn_mean, bn_var, w2, b2):
    x = np.asarray(x, dtype=np.float32)
    scale = np.asarray(bn_gamma) / np.sqrt(np.asarray(bn_var) + EPS)
    w1s = (np.asarray(w1) * scale[:, None]).astype(np.float32)
    b1f = (np.asarray(b1) * scale + np.asarray(bn_beta)
           - np.asarray(bn_mean) * scale).astype(np.float32)
    w1sT = np.ascontiguousarray(w1s.T.astype(np.float16))        # [128, 32]
    w2T = np.ascontiguousarray(np.asarray(w2, np.float32).T.astype(np.float16))
    b1fc = np.ascontiguousarray(b1f[:, None].astype(np.float32))
    b2fc = np.ascontiguousarray(np.asarray(b2, np.float32)[:, None])

    # W-edge mask: kerm[o, p] = 0 where w + dj leaves the row
    wcol = np.arange(P, dtype=np.int64) % W
    maskt = np.zeros((49, P), dtype=np.float16)
    for ipp in range(KK):
        for jpp in range(KK):
            dj = jpp - 3
            maskt[ipp * KK + jpp] = ((wcol + dj >= 0) & (wcol + dj < W))
    maskt = np.ascontiguousarray(maskt)

    # one-hot stationaries: eye[k, o*128+m] = (k == o)
    eye = np.zeros((49, 49 * 128), dtype=np.float16)
    for o in range(49):
        eye[o, o * 128:(o + 1) * 128] = 1.0
    eye = np.ascontiguousarray(eye)

    in_maps = []
    for core in range(8):
        b, half = divmod(core, 2)
        h0 = HH * half
        xw = np.zeros((C, NIN), dtype=np.float16)
        lo = max(0, h0 - 3)
        hi = min(H, h0 + HH + 3)
        src = x[b, :, lo:hi, :].reshape(C, -1).astype(np.float16)
        start = 4 + W * (lo - h0 + 3)
        xw[:, start:start + src.shape[1]] = src
        xwo = np.zeros((C, NIN), dtype=np.float16)
        xwo[:, :NIN - 1] = xw[:, 1:]
        in_maps.append({
            "xwe": xw, "xwo": xwo, "w1sT": w1sT, "b1f": b1fc, "w2T": w2T,
            "b2f": b2fc, "maskt": maskt, "eye": eye,
        })
    return in_maps


def run(inputs: dict, trace: bool = False):
    from concourse.bass_utils import run_bass_kernel_spmd

    nc = _get_nc()
    in_maps = _host_prep(**inputs)
    res = run_bass_kernel_spmd(
        nc, in_maps, core_ids=list(range(8)), trace=trace,
    )
    out = np.zeros((B, C, H, W), dtype=np.float32)
    for core in range(8):
        b, half = divmod(core, 2)
        h0 = HH * half
        out[b, :, h0:h0 + HH, :] = (
            res.results[core]["out"].astype(np.float32).reshape(C, HH, W)
        )
    return out, res


def kernel(**inputs) -> np.ndarray:
    out, _ = run(inputs, trace=False)
    return out
